# revision 38
# baseline (speedup 1.0000x reference)
"""2-layer GAT + MLP head on 8 TRN2 NeuronCores.

Strategy (dst-sharded, software-pipelined):
- Nodes padded to NP=20480; each core owns a contiguous 2560-dst shard.
- Edges (incl. self-loops, PyG mean-fill edge attr) sorted by dst,
  grouped into 128-dst tiles, padded per tile-slot to chunk counts
  shared by all cores (SPMD: one program).
- Per layer a node table [NP, 640] bf16 in HBM: cols [0:512) = h
  (head-interleaved (c,h) order), bytes [1024:1056) = asrc|adst (f32
  bits). The layer-1 table is split into two halves, each assembled by
  its own Shared-HBM AllGather; the first fires mid layer-0 so it
  hides, and layer-1 aggregation runs two passes (src-half A then B,
  partial sums parked in SBUF) so it starts before the second
  AllGather lands.
- Aggregation per 128-edge chunk: gather rows by src (4 queue-split
  DMAs per 8-chunk super), fp8 one-hot blocks ([e,d] + [d,e]) streamed
  as one fused ohz tensor; p = exp(lrelu(asrc+adst+aedge)) with adst
  expanded via one-hot matmul; out[dst] += (p*h) via one-hot matmul in
  PSUM; softmax denominator via a second matmul with rhs=p.
- Pipelining: gathers issued 2 supers ahead, alpha chains 1 super
  ahead, tile finalize deferred into the next tile so the in-order
  engines never stall on cross-engine chains.
- dst-tile alphas for layer 1 captured into SBUF during fin0 (no
  gather); layer-0 ones via one batched 2560-row gather from H0.
"""

import numpy as np
import ml_dtypes

import concourse.bacc as bacc
import concourse.bass as bass
import concourse.mybir as mybir
import concourse.tile as tile
from concourse.bass_utils import run_bass_kernel_spmd

F32 = mybir.dt.float32
F8 = mybir.dt.float8e4
BF16 = mybir.dt.bfloat16
I16 = mybir.dt.int16
AF = mybir.ActivationFunctionType
OP = mybir.AluOpType

NCORES = 8
SCC = 8  # chunks (of 128 edges) per gather super-chunk


def _bcastI(ap_tile, j, reps):
    """[128, SCC, 4] tile -> [128, reps, 4] broadcast AP of slot j with the
    head dim last at unit stride (head-interleaved gp layout)."""
    sl = ap_tile[:, j, :]
    return bass.AP(sl.tensor, sl.offset, [list(sl.ap[0]), [0, reps], list(sl.ap[-1])])


def _build_program(NP, F_IN, HC, H, C, NT, K0, KA, KB, NTA, FTS,
                   use_b0, use_b1, use_l0b, use_l1b):
    NC0 = int(sum(K0))
    NC1 = int(sum(KA) + sum(KB))
    SW0 = NC0 * 8
    SW1 = NC1 * 8
    TW = HC + 128  # bf16 table row: h | asrc,adst (f32 bits) | pad
    KB_ = HC // 128
    NTB = NT - NTA
    RA = NCORES * NTA * 128          # rows in table half A
    RB = NP - RA
    SA = int(sum(KA)) // SCC         # segment-A supers (padded to %SCC)
    MT = NP // 128

    t_of_q0 = []
    for t in range(NT):
        t_of_q0 += [t] * K0[t]
    t_of_q1 = []
    for t in range(NT):
        t_of_q1 += [t] * KA[t]
    for t in range(NT):
        t_of_q1 += [t] * KB[t]

    nc = bacc.Bacc(dynamic_dma_scratch_size=65536, num_swdge_queues=4)
    P = nc.declare_dram_parameter

    xT = P("xT", [F_IN, NP], BF16, isOutput=False)
    r0h = P("r0h", [F_IN, HC], BF16, isOutput=False)
    r0a = P("r0a", [F_IN, 8], BF16, isOutput=False)
    r1h = P("r1h", [HC, HC], BF16, isOutput=False)
    r1a = P("r1a", [HC, 8], BF16, isOutput=False)
    r2 = P("r2", [HC, FTS], F32, isOutput=False)
    r3 = P("r3", [FTS, 1], F32, isOutput=False)
    ident = P("ident", [128, 128], F32, isOutput=False)
    identb = P("identb", [128, 128], BF16, isOutput=False)
    srcw0 = P("srcw0", [128, SW0], I16, isOutput=False)
    srcw1 = P("srcw1", [128, SW1], I16, isOutput=False)
    trw = P("trw", [128, NT * 8], I16, isOutput=False)
    ohz0 = P("ohz0", [128, NC0, 256], F8, isOutput=False)
    ohz1 = P("ohz1", [128, NC1, 256], F8, isOutput=False)
    ae0 = P("ae0", [128, NC0, 4], F32, isOutput=False)
    ae1 = P("ae1", [128, NC1, 4], F32, isOutput=False)
    if use_b0:
        b0t = P("b0t", [128, HC], F32, isOutput=False)
    if use_b1:
        b1t = P("b1t", [128, HC], F32, isOutput=False)
    if use_l0b:
        l0bt = P("l0bt", [128, FTS], F32, isOutput=False)
    if use_l1b:
        l1bt = P("l1bt", [128, 1], F32, isOutput=False)
    outp = P("out", [NT * 128, 1], F32, isOutput=True)

    with tile.TileContext(nc) as tc:
        with (
            tc.tile_pool(name="const", bufs=1) as const,
            tc.tile_pool(name="stage", bufs=2) as stage,
            tc.tile_pool(name="work", bufs=3) as work,
            tc.tile_pool(name="tp", bufs=5) as tp,
            tc.tile_pool(name="adp", bufs=1) as adp,
            tc.tile_pool(name="psacc", bufs=3, space="PSUM") as psacc,
            tc.tile_pool(name="pss", bufs=2, space="PSUM") as pss,
            tc.tile_pool(name="pstr", bufs=3, space="PSUM") as pstr,
            tc.tile_pool(name="dram", bufs=1, space="DRAM") as dram,
        ):
            H0 = dram.tile([NP, TW], BF16, tag="H0")
            H1A = dram.tile([RA, TW], BF16, tag="H1A", addr_space="Shared")
            H1B = dram.tile([RB, TW], BF16, tag="H1B", addr_space="Shared")
            H1g = [dram.tile([(NTA if g == 0 else NTB) * 128, TW], BF16,
                             tag=f"H1g{g}", name=f"H1g{g}") for g in range(2)]

            _cn = [0]

            def cload(ap_in, shape, dt=F32, tag=None):
                _cn[0] += 1
                cname = tag or f"c{_cn[0]}"
                t = const.tile(shape, dt, tag=cname, name=f"{cname}_{_cn[0]}")
                nc.sync.dma_start(out=t[:], in_=ap_in)
                return t

            r0h_s = cload(r0h[:, :], [F_IN, HC], BF16)
            r0a_s = cload(r0a[:, :], [F_IN, 8], BF16)
            r1h_s = [cload(r1h[k * 128:(k + 1) * 128, :], [128, HC], BF16)
                     for k in range(KB_)]
            r1a_s = [cload(r1a[k * 128:(k + 1) * 128, :], [128, 8], BF16)
                     for k in range(KB_)]
            r2_s = [cload(r2[k * 128:(k + 1) * 128, :], [128, FTS])
                    for k in range(KB_)]
            r3_s = cload(r3[:, :], [FTS, 1])
            id_s = cload(ident[:, :], [128, 128])
            idb_s = cload(identb[:, :], [128, 128], BF16)
            srcw0_s = cload(srcw0[:, :], [128, SW0], I16, tag="srcw")
            trw_s = cload(trw[:, :], [128, NT * 8], I16)
            ae0_s = cload(ae0[:, :, :], [128, NC0, 4], tag="ae")
            b0_s = cload(b0t[:, :], [128, HC]) if use_b0 else None
            b1_s = cload(b1t[:, :], [128, HC]) if use_b1 else None
            l0b_s = cload(l0bt[:, :], [128, FTS]) if use_l0b else None
            l1b_s = cload(l1bt[:, :], [128, 1]) if use_l1b else None

            # ---- phase A: layer-0 table (xT pre-permuted to row order) ----
            nc._state.push_named_scope("phaseA")
            NQ4 = NP // 4
            MQ = MT // 4
            xq = [None] * 4

            def load_xq(qq):
                if qq >= 4:
                    return
                xq[qq] = stage.tile([F_IN, NQ4], BF16, tag="xT",
                                    name=f"xq{qq}", bufs=2)
                eng = nc.sync if qq < 2 else nc.scalar
                eng.dma_start(out=xq[qq][:],
                              in_=xT[:, qq * NQ4:(qq + 1) * NQ4])

            load_xq(0)
            load_xq(1)
            st2 = None
            for mr in range(MT):
                if mr % MQ == MQ - 1:
                    load_xq(mr // MQ + 2)
                lx = xq[mr // MQ][:, (mr % MQ) * 128:(mr % MQ + 1) * 128]
                pool = psacc if mr % 2 == 0 else pstr
                ph = pool.tile([128, HC], F32,
                               tag="ph" if mr % 2 == 0 else "pt", name="phA")
                nc.tensor.matmul(ph[:], lx, r0h_s[:], start=True, stop=True)
                pa = pss.tile([128, 8], F32, tag="pss")
                nc.tensor.matmul(pa[:], lx, r0a_s[:], start=True, stop=True)
                half = mr % 2
                if half == 0:
                    st2 = stage.tile([128, 2, TW], BF16, tag="hrow2", bufs=4)
                if half == 0:
                    nc.vector.tensor_copy(st2[:, half, 0:HC], ph[:])
                    nc.vector.tensor_copy(
                        st2[:, half, HC:HC + 16].bitcast(F32), pa[:])
                else:
                    nc.scalar.activation(st2[:, half, 0:HC], ph[:], AF.Copy)
                    nc.vector.tensor_copy(
                        st2[:, half, HC:HC + 16].bitcast(F32), pa[:])
                if half == 1:
                    dst = H0[(mr - 1) * 128:(mr + 1) * 128, :].rearrange(
                        "(j p) c -> p j c", p=128)
                    eng = nc.sync if (mr // 2) % 2 == 0 else nc.scalar
                    eng.dma_start(out=dst, in_=st2[:])
            nc._state.pop_named_scope("phaseA")

            # ---- layer-0 alpha_dst for own dst tiles: one batched gather ----
            adt_all = adp.tile([128, NT, 128], BF16, tag="adta")
            nc.gpsimd.dma_gather(
                adt_all[:], H0[:, HC:TW], trw_s[:, 0:NT * 8],
                NT * 128, NT * 128, 128, elem_step=TW,
                single_packet=False, queue_num=3)
            adts0 = adp.tile([128, NT, 4], BF16, tag="adts0")
            nc.vector.tensor_copy(adts0[:], adt_all[:, :, 8:16].bitcast(F32))

            # persistent alpha_dst for layer-1's own dst tiles (filled in fin0)
            adts1 = adp.tile([128, NT, 4], BF16, tag="adts1")
            # SBUF partial-sum parking for layer-1's two-pass aggregation
            accO = adp.tile([128, NT, HC], F32, tag="adta")
            accS = adp.tile([128, NT, 4], F32, tag="accS")

            # ---- aggregation over one layer's edges ----
            def agg_layer(tbl_of_s, srcw_s, ohz_p, ae_s, adts, t_of_q,
                          NCHUNK, passes, fin_pre, fin_post, acc_last):
                NSUP = -(-NCHUNK // SCC)
                gstate = {}

                def nch_of(s):
                    return min(SCC, NCHUNK - s * SCC)

                def issue_gather(s):
                    if s >= NSUP:
                        return
                    nch = nch_of(s)
                    tbl = tbl_of_s(s)
                    gA = stage.tile([128, SCC, TW], BF16, tag="gA",
                                    name="gA", bufs=4)
                    c0 = s * SCC * 8
                    base = 0
                    nq = min(4, nch)
                    for qi in range(nq):
                        take = (nch - base + (nq - qi) - 1) // (nq - qi)
                        nc.gpsimd.dma_gather(
                            gA[:, base:base + take, :], tbl,
                            srcw_s[:, c0 + base * 8:c0 + (base + take) * 8],
                            take * 128, take * 128, TW,
                            single_packet=False, queue_num=qi)
                        base += take
                    oz = stage.tile([128, SCC, 256], F8, tag="ohz",
                                    name="oz", bufs=3)
                    nc.sync.dma_start(
                        out=oz[:, 0:nch, :],
                        in_=ohz_p[:, s * SCC:s * SCC + nch, :])
                    gstate[s] = [gA, oz, None, None]

                def emit_pead(s):
                    if s >= NSUP:
                        return
                    nch = nch_of(s)
                    oz = gstate[s][1]
                    pead = pstr.tile([128, SCC * 4], F32, tag="pt", name="pead")
                    for jj in range(nch):
                        q = s * SCC + jj
                        nc.tensor.matmul(
                            pead[:, jj * 4:(jj + 1) * 4],
                            oz[:, jj, 128:256], adts[:, t_of_q[q], :],
                            start=True, stop=True)
                    gstate[s][2] = pead

                def emit_alpha(s):
                    if s >= NSUP:
                        return
                    nch = nch_of(s)
                    gA, oz, pead, _ = gstate[s]
                    asrc = gA[:, 0:nch, HC:HC + 8].bitcast(F32)
                    t0 = work.tile([128, SCC, 4], F32, tag="t0", bufs=2)
                    nc.vector.tensor_add(
                        t0[:, 0:nch, :], asrc,
                        ae_s[:, s * SCC:s * SCC + nch, :])
                    t1 = work.tile([128, SCC, 4], F32, tag="t1", bufs=2)
                    nc.vector.tensor_add(
                        t1[:, 0:nch, :], t0[:, 0:nch, :],
                        pead[:, 0:nch * 4].rearrange("x (a b) -> x a b", b=4))
                    t2 = work.tile([128, SCC, 4], F32, tag="t2", bufs=2)
                    nc.scalar.activation(
                        t2[:, 0:nch, :], t1[:, 0:nch, :], AF.Copy, scale=0.2)
                    tl = work.tile([128, SCC, 4], F32, tag="tl", bufs=2)
                    nc.vector.tensor_max(
                        tl[:, 0:nch, :], t1[:, 0:nch, :], t2[:, 0:nch, :])
                    pf = work.tile([128, SCC, 4], F32, tag="pf", bufs=3)
                    nc.scalar.activation(pf[:, 0:nch, :], tl[:, 0:nch, :],
                                         AF.Exp)
                    pb = work.tile([128, SCC, 4], BF16, tag="p", bufs=3)
                    nc.scalar.activation(pb[:, 0:nch, :], pf[:, 0:nch, :],
                                         AF.Copy)
                    gstate[s][3] = pb

                issue_gather(0)
                issue_gather(1)
                emit_pead(0)
                emit_alpha(0)

                pend = None
                q = 0
                last_pass = len(passes) - 1
                for pi, seg in enumerate(passes):
                    final = (pi == last_pass)
                    for (t, cnt) in seg:
                        ps_o = psacc.tile([128, HC], F32, tag="ph")
                        ps_s = pss.tile([128, 8], F32, tag="pss")
                        fin_t, fin_ar = None, None
                        for k in range(cnt):
                            s, j = divmod(q, SCC)
                            if j == 0:
                                issue_gather(s + 2)
                                emit_pead(s + 1)
                            if j == 2:
                                emit_alpha(s + 1)
                            if final and pend is not None and \
                                    k == min(2, cnt - 1):
                                fin_t = pend[0]
                                fin_ar = fin_pre(*pend)
                                pend = None
                            gA, oz, pead, pb = gstate[s]
                            gp = work.tile([128, HC], BF16, tag="gp", bufs=4)
                            nc.vector.tensor_mul(
                                gp[:].rearrange("x (c h) -> x c h", h=H),
                                gA[:, j, 0:HC].rearrange(
                                    "x (c h) -> x c h", h=H),
                                _bcastI(pb, j, C))
                            first, last = (k == 0), (k == cnt - 1)
                            oh_j = oz[:, j, 0:128]
                            nc.tensor.matmul(ps_o[:], oh_j, gp[:],
                                             start=first, stop=last)
                            nc.tensor.matmul(ps_s[:, 0:4], oh_j, pb[:, j, :],
                                             start=first, stop=last)
                            q += 1
                        if not final:
                            # park partial sums in SBUF
                            if t % 2 == 0:
                                nc.vector.tensor_copy(accO[:, t, :], ps_o[:])
                            else:
                                nc.scalar.activation(accO[:, t, :], ps_o[:],
                                                     AF.Copy)
                            nc.vector.tensor_copy(accS[:, t, :], ps_s[:, 0:4])
                        else:
                            if fin_ar is not None:
                                fin_post(fin_t, fin_ar)
                            pend = (t, ps_o, ps_s)
                ar = fin_pre(*pend)
                fin_post(pend[0], ar)

            # ---- tile finalize: softmax-normalize + relu (fused on scalar) --
            def norm_relu(ps_o, ps_s, acc_t, bias_s, out_dt):
                if acc_t is not None:
                    so = work.tile([128, HC], F32, tag="so", bufs=1)
                    nc.vector.tensor_add(so[:], ps_o[:], accO[:, acc_t, :])
                    ss = work.tile([128, 4], F32, tag="ss", bufs=2)
                    nc.vector.tensor_add(ss[:], ps_s[:, 0:4],
                                         accS[:, acc_t, :])
                else:
                    so, ss = ps_o, ps_s[:, 0:4]
                sp = work.tile([128, 4], F32, tag="sp")
                nc.vector.tensor_scalar_add(sp[:], ss[:], 1e-16)
                rc = work.tile([128, 4], F32, tag="rc")
                nc.vector.reciprocal(rc[:], sp[:])
                if bias_s is None:
                    ar = work.tile([128, HC], out_dt,
                                   tag=f"ar{out_dt}", bufs=2)
                    arv = ar[:].rearrange("x (c h) -> x h c", h=H)
                    psv = so[:].rearrange("x (c h) -> x h c", h=H)
                    for h in range(H):
                        nc.scalar.activation(
                            arv[:, h, :], psv[:, h, :],
                            AF.Relu, scale=rc[:, h:h + 1])
                    return ar
                ao = work.tile([128, HC], F32, tag="ao", bufs=2)
                aov = ao[:].rearrange("x (c h) -> x h c", h=H)
                psv = so[:].rearrange("x (c h) -> x h c", h=H)
                for h in range(H):
                    nc.vector.tensor_scalar_mul(
                        aov[:, h, :], psv[:, h, :], rc[:, h:h + 1])
                ab = work.tile([128, HC], F32, tag="ao", bufs=2)
                nc.vector.tensor_add(ab[:], ao[:], bias_s[:])
                ar = work.tile([128, HC], out_dt, tag=f"ar{out_dt}", bufs=2)
                nc.scalar.activation(ar[:], ab[:], AF.Relu)
                return ar

            # ---- layer-0 finalize: transpose + layer-1 linear + half AG ----
            def fin_pre0(t, ps_o, ps_s):
                return norm_relu(ps_o, ps_s, None, b0_s, BF16)

            def fin_post0(t, ar):
                a0k = []
                for kk in range(KB_):
                    pt = pstr.tile([128, 128], BF16, tag="pt", name="ptb")
                    nc.tensor.transpose(pt[:], ar[:, kk * 128:(kk + 1) * 128],
                                        idb_s[:])
                    ak = tp.tile([128, 128], BF16, tag="a1T", name=f"a0k{kk}")
                    if kk % 2 == 0:
                        nc.scalar.activation(ak[:], pt[:], AF.Copy)
                    else:
                        nc.vector.tensor_copy(ak[:], pt[:])
                    a0k.append(ak)
                ph1 = psacc.tile([128, HC], F32, tag="ph")
                pa1 = pss.tile([128, 8], F32, tag="pss")
                for kk in range(KB_):
                    first, last = (kk == 0), (kk == KB_ - 1)
                    nc.tensor.matmul(ph1[:], a0k[kk][:], r1h_s[kk][:],
                                     start=first, stop=last)
                    nc.tensor.matmul(pa1[:], a0k[kk][:], r1a_s[kk][:],
                                     start=first, stop=last)
                st = stage.tile([128, TW], BF16, tag="hrow", bufs=3)
                if t % 2 == 0:
                    nc.vector.tensor_copy(st[:, 0:HC], ph1[:])
                else:
                    nc.scalar.activation(st[:, 0:HC], ph1[:], AF.Copy)
                nc.vector.tensor_copy(st[:, HC:HC + 16].bitcast(F32), pa1[:])
                nc.vector.tensor_copy(adts1[:, t, :], pa1[:, 4:8])
                g = 0 if t < NTA else 1
                loc = t - g * NTA
                nc.sync.dma_start(out=H1g[g][loc * 128:(loc + 1) * 128, :],
                                  in_=st[:])
                if loc == (NTA if g == 0 else NTB) - 1:
                    nc.gpsimd.collective_compute(
                        "AllGather", OP.bypass,
                        replica_groups=[list(range(NCORES))],
                        ins=[H1g[g].opt()],
                        outs=[(H1A if g == 0 else H1B)[:, :].opt()],
                    )

            nc._state.push_named_scope("phaseB")
            agg_layer(lambda s: H0[:, :], srcw0_s, ohz0, ae0_s, adts0,
                      t_of_q0, NC0, [[(t, K0[t]) for t in range(NT)]],
                      fin_pre0, fin_post0, None)
            nc._state.pop_named_scope("phaseB")

            # ---- layer-1 aggregation + MLP head per dst tile ----
            def fin_pre1(t, ps_o, ps_s):
                return norm_relu(ps_o, ps_s, t, b1_s, F32)

            def fin_post1(t, ar):
                h2p = psacc.tile([128, FTS], F32, tag="ph")
                for kk in range(KB_):
                    pt = pstr.tile([128, 128], F32, tag="pt", name="ptf")
                    nc.tensor.transpose(pt[:], ar[:, kk * 128:(kk + 1) * 128],
                                        id_s[:])
                    a1k = tp.tile([128, 128], F32, tag="a1T")
                    nc.vector.tensor_copy(a1k[:], pt[:])
                    nc.tensor.matmul(h2p[:], a1k[:], r2_s[kk][:],
                                     start=(kk == 0), stop=(kk == KB_ - 1))
                if use_l0b:
                    h2b = work.tile([128, FTS], F32, tag="h2b")
                    nc.vector.tensor_add(h2b[:], h2p[:], l0b_s[:])
                else:
                    h2b = h2p
                h2r = work.tile([128, FTS], F32, tag="h2r")
                nc.scalar.activation(h2r[:], h2b[:], AF.Relu)
                pt2 = pstr.tile([128, 128], F32, tag="pt")
                nc.tensor.transpose(pt2[:], h2r[:], id_s[:])
                h2T = tp.tile([128, 128], F32, tag="a1T")
                nc.vector.tensor_copy(h2T[:], pt2[:])
                po = pss.tile([128, 8], F32, tag="pss")
                nc.tensor.matmul(po[:, 0:1], h2T[:], r3_s[:],
                                 start=True, stop=True)
                ob = work.tile([128, 1], F32, tag="ob")
                if use_l1b:
                    nc.vector.tensor_add(ob[:], po[:, 0:1], l1b_s[:])
                else:
                    nc.vector.tensor_copy(ob[:], po[:, 0:1])
                nc.sync.dma_start(out=outp[t * 128:(t + 1) * 128, :], in_=ob[:])

            nc._state.push_named_scope("phaseD")
            srcw1_s = cload(srcw1[:, :], [128, SW1], I16, tag="srcw")
            ae1_s = cload(ae1[:, :, :], [128, NC1, 4], tag="ae")
            agg_layer(lambda s: H1A[:, :] if s < SA else H1B[:, :],
                      srcw1_s, ohz1, ae1_s, adts1, t_of_q1, NC1,
                      [[(t, KA[t]) for t in range(NT)],
                       [(t, KB[t]) for t in range(NT)]],
                      fin_pre1, fin_post1, True)
            nc._state.pop_named_scope("phaseD")

    nc.finalize()
    return nc


def _wrap_idx(v, E_pad):
    blk = np.zeros((16, E_pad // 16), np.int16)
    ar = np.arange(E_pad)
    blk[ar % 16, ar // 16] = v.astype(np.int16)
    return np.tile(blk, (8, 1))


def kernel(x, edge_index, edge_weights,
           W0, as0, ad0, We0, ae0, b0,
           W1, as1, ad1, We1, ae1, b1,
           L0W, L0b, L1W, L1b):
    x = np.asarray(x, np.float32)
    N, F_IN = x.shape
    HC = W0.shape[0]
    H, C = np.asarray(as0).shape
    FTS = np.asarray(L0W).shape[0]

    NT = -(-N // (128 * NCORES))
    NTA = NT // 2
    NTB = NT - NTA
    SHARD = NT * 128
    NP = SHARD * NCORES
    RA = NCORES * NTA * 128

    # table-row permutation: half-major (A=tiles<NTA), core-interleaved within
    nodes = np.arange(NP)
    core = nodes // SHARD
    rr = nodes % SHARD
    tt = rr // 128
    gg = (tt >= NTA).astype(np.int64)
    off = (tt - gg * NTA) * 128 + rr % 128
    stride = np.where(gg == 0, NTA * 128, NTB * 128)
    t_of_n = gg * RA + core * stride + off            # node -> table row

    # ---- edges ----
    ew_in = np.asarray(edge_weights, np.float32)
    src = np.concatenate([np.asarray(edge_index[0]), np.arange(N)])
    dst = np.concatenate([np.asarray(edge_index[1]), np.arange(N)])
    ew = np.concatenate([ew_in, np.full(N, ew_in.mean(), np.float32)])
    order = np.argsort(dst, kind="stable")
    src_s, dst_s, ew_s = src[order], dst[order], ew[order]
    srow = t_of_n[src_s]                              # src table row

    NTG = NP // 128
    tile_of = (dst_s // 128).astype(np.int64)
    tcounts = np.bincount(tile_of, minlength=NTG)
    tstart = np.concatenate([[0], np.cumsum(tcounts)])

    # layer-0 chunk counts (per dst tile, max over cores)
    K0 = [max(1, int(max(-(-tcounts[i * NT + t] // 128)
                         for i in range(NCORES)))) for t in range(NT)]
    NC0 = int(sum(K0))

    # layer-1 per-tile per-src-half counts
    cA = np.zeros((NCORES, NT), np.int64)
    cB = np.zeros((NCORES, NT), np.int64)
    for i in range(NCORES):
        for t in range(NT):
            g = i * NT + t
            sl = slice(tstart[g], tstart[g] + int(tcounts[g]))
            inA = srow[sl] < RA
            cA[i, t] = int(inA.sum())
            cB[i, t] = int((~inA).sum())
    KA = [max(1, int(max(-(-cA[i, t] // 128) for i in range(NCORES))))
          for t in range(NT)]
    KB = [max(1, int(max(-(-cB[i, t] // 128) for i in range(NCORES))))
          for t in range(NT)]
    KA[NT - 1] += (-sum(KA)) % SCC     # segment-A supers end on a boundary
    NC1 = int(sum(KA) + sum(KB))

    # ---- weight folding (host, O(weights)) ----
    as0 = np.asarray(as0, np.float32)
    ad0 = np.asarray(ad0, np.float32)
    ae0w = np.asarray(ae0, np.float32)
    as1 = np.asarray(as1, np.float32)
    ad1 = np.asarray(ad1, np.float32)
    ae1w = np.asarray(ae1, np.float32)
    W0 = np.asarray(W0, np.float32)
    W1 = np.asarray(W1, np.float32)
    We0 = np.asarray(We0, np.float32)
    We1 = np.asarray(We1, np.float32)

    k0 = (We0.reshape(H, C) * ae0w).sum(1).astype(np.float32)
    k1 = (We1.reshape(H, C) * ae1w).sum(1).astype(np.float32)

    def fold(W, a):
        blk = np.zeros((HC, H), np.float32)
        for h in range(H):
            blk[h * C:(h + 1) * C, h] = a[h]
        return (W.T @ blk).astype(np.float32)

    bf = ml_dtypes.bfloat16
    f8 = ml_dtypes.float8_e4m3fn
    # head-interleaved column order: table col c*H+h holds head h channel c
    iperm = np.asarray([h * C + c for c in range(C) for h in range(H)])
    r0h = W0.T[:, iperm].astype(bf)
    r0a = np.concatenate([fold(W0, as0), fold(W0, ad0)], 1).astype(bf)
    r1h = W1.T[iperm][:, iperm].astype(bf)
    r1a = np.concatenate([fold(W1, as1), fold(W1, ad1)], 1)[iperm].astype(bf)
    r2 = np.asarray(L0W, np.float32).T[iperm].copy()
    r3 = np.asarray(L1W, np.float32).T.copy()

    # xT in TABLE-ROW order: column r of xT = x[node(r)]
    inv = np.empty(NP, np.int64)
    inv[t_of_n] = nodes                              # table row -> node
    xa = np.zeros((NP, F_IN), np.float32)
    xa[:N] = x
    xT = np.ascontiguousarray(xa[inv].T).astype(bf)

    ident = np.eye(128, dtype=np.float32)
    identb = np.eye(128, dtype=np.float32).astype(bf)

    use_b0 = bool(np.any(b0))
    use_b1 = bool(np.any(b1))
    use_l0b = bool(np.any(np.asarray(L0b)))
    use_l1b = bool(np.any(np.asarray(L1b)))

    def build_oh(dlocp, nchunk):
        ohcube = np.zeros((nchunk, 128, 128), np.float32)  # [q, e, d]
        dl2 = dlocp.reshape(nchunk, 128)
        qs, es = np.nonzero(dl2 >= 0)
        ohcube[qs, es, dl2[qs, es]] = 1.0
        ohz_np = np.empty((128, nchunk, 256), f8)
        ohz_np[:, :, 0:128] = ohcube.transpose(1, 0, 2).astype(f8)
        ohz_np[:, :, 128:256] = ohcube.transpose(2, 0, 1).astype(f8)
        return ohz_np

    in_maps = []
    for i in range(NCORES):
        # ---- layer-0 edge layout: per-tile contiguous ----
        E0 = NC0 * 128
        srcp0 = np.zeros(E0, np.int64)
        dloc0 = np.full(E0, -1, np.int64)
        ewp0 = np.zeros(E0, np.float32)
        offq = 0
        for t in range(NT):
            g = i * NT + t
            cnt = int(tcounts[g])
            sl = slice(tstart[g], tstart[g] + cnt)
            srcp0[offq:offq + cnt] = srow[sl]
            dloc0[offq:offq + cnt] = dst_s[sl] - g * 128
            ewp0[offq:offq + cnt] = ew_s[sl]
            offq += K0[t] * 128
        # ---- layer-1 edge layout: [tiles x half-A | tiles x half-B] ----
        E1 = NC1 * 128
        srcp1 = np.zeros(E1, np.int64)
        dloc1 = np.full(E1, -1, np.int64)
        ewp1 = np.zeros(E1, np.float32)
        offq = 0
        for seg in (0, 1):
            Ks = KA if seg == 0 else KB
            for t in range(NT):
                g = i * NT + t
                sl = slice(tstart[g], tstart[g] + int(tcounts[g]))
                inA = srow[sl] < RA
                pick = inA if seg == 0 else ~inA
                cnt = int(pick.sum())
                rows = srow[sl][pick]
                if seg == 1:
                    rows = rows - RA
                srcp1[offq:offq + cnt] = rows
                dloc1[offq:offq + cnt] = dst_s[sl][pick] - g * 128
                ewp1[offq:offq + cnt] = ew_s[sl][pick]
                offq += Ks[t] * 128
        ae0p = (ewp0[:, None] * k0[None, :]).reshape(NC0, 128, 4)
        ae1p = (ewp1[:, None] * k1[None, :]).reshape(NC1, 128, 4)
        # own dst-tile table rows for the layer-0 alpha_dst gather
        trows = np.empty((NT, 128), np.int64)
        for t in range(NT):
            base = t_of_n[i * SHARD + t * 128]
            trows[t] = base + np.arange(128)
        im = {
            "xT": xT, "r0h": r0h, "r0a": r0a, "r1h": r1h, "r1a": r1a,
            "r2": r2, "r3": r3, "ident": ident, "identb": identb,
            "srcw0": _wrap_idx(srcp0, E0), "srcw1": _wrap_idx(srcp1, E1),
            "trw": _wrap_idx(trows.reshape(-1), NT * 128),
            "ohz0": build_oh(dloc0, NC0), "ohz1": build_oh(dloc1, NC1),
            "ae0": np.ascontiguousarray(ae0p.transpose(1, 0, 2)),
            "ae1": np.ascontiguousarray(ae1p.transpose(1, 0, 2)),
        }
        if use_b0:
            im["b0t"] = np.tile(np.asarray(b0, np.float32)[iperm][None, :],
                                (128, 1))
        if use_b1:
            im["b1t"] = np.tile(np.asarray(b1, np.float32)[iperm][None, :],
                                (128, 1))
        if use_l0b:
            im["l0bt"] = np.tile(np.asarray(L0b, np.float32)[None, :],
                                 (128, 1))
        if use_l1b:
            im["l1bt"] = np.tile(np.asarray(L1b, np.float32).reshape(1, 1),
                                 (128, 1))
        in_maps.append(im)

    nc = _build_program(NP, F_IN, HC, H, C, NT, K0, KA, KB, NTA, FTS,
                        use_b0, use_b1, use_l0b, use_l1b)
    res = run_bass_kernel_spmd(nc, in_maps, list(range(NCORES)))
    out = np.concatenate([res.results[i]["out"][:, 0] for i in range(NCORES)])
    return out[:N].astype(np.float32)


# revision 40
# speedup vs baseline: 1.0068x; 1.0068x over previous
"""2-layer GAT + MLP head on 8 TRN2 NeuronCores.

Strategy (dst-sharded, software-pipelined):
- Nodes padded to NP=20480; each core owns a contiguous 2560-dst shard.
- Edges (incl. self-loops, PyG mean-fill edge attr) sorted by dst,
  grouped into 128-dst tiles, padded per tile-slot to chunk counts
  shared by all cores (SPMD: one program).
- Per layer a node table [NP, 640] bf16 in HBM: cols [0:512) = h
  (head-interleaved (c,h) order), bytes [1024:1056) = asrc|adst (f32
  bits). The layer-1 table is split into two halves, each assembled by
  its own Shared-HBM AllGather; the first fires mid layer-0 so it
  hides, and layer-1 aggregation runs two passes (src-half A then B,
  partial sums parked in SBUF) so it starts before the second
  AllGather lands.
- Aggregation per 128-edge chunk: gather rows by src (4 queue-split
  DMAs per 8-chunk super), fp8 one-hot blocks ([e,d] + [d,e]) streamed
  as one fused ohz tensor; p = exp(lrelu(asrc+adst+aedge)) with adst
  expanded via one-hot matmul; out[dst] += (p*h) via one-hot matmul in
  PSUM; softmax denominator via a second matmul with rhs=p.
- Pipelining: gathers issued 2 supers ahead, alpha chains 1 super
  ahead, tile finalize deferred into the next tile so the in-order
  engines never stall on cross-engine chains.
- dst-tile alphas for layer 1 captured into SBUF during fin0 (no
  gather); layer-0 ones via one batched 2560-row gather from H0.
"""

import numpy as np
import ml_dtypes

import concourse.bacc as bacc
import concourse.bass as bass
import concourse.mybir as mybir
import concourse.tile as tile
from concourse.bass_utils import run_bass_kernel_spmd

F32 = mybir.dt.float32
F8 = mybir.dt.float8e4
BF16 = mybir.dt.bfloat16
I16 = mybir.dt.int16
AF = mybir.ActivationFunctionType
OP = mybir.AluOpType

NCORES = 8
SCC = 8  # chunks (of 128 edges) per gather super-chunk


def _bcastI(ap_tile, j, reps):
    """[128, SCC, 4] tile -> [128, reps, 4] broadcast AP of slot j with the
    head dim last at unit stride (head-interleaved gp layout)."""
    sl = ap_tile[:, j, :]
    return bass.AP(sl.tensor, sl.offset, [list(sl.ap[0]), [0, reps], list(sl.ap[-1])])


def _build_program(NP, F_IN, HC, H, C, NT, K0, KA, KB, NTA, FTS,
                   use_b0, use_b1, use_l0b, use_l1b):
    NC0 = int(sum(K0))
    NC1 = int(sum(KA) + sum(KB))
    SW0 = NC0 * 8
    SW1 = NC1 * 8
    TW = HC + 128  # bf16 table row: h | asrc,adst (f32 bits) | pad
    KB_ = HC // 128
    NTB = NT - NTA
    RA = NCORES * NTA * 128          # rows in table half A
    RB = NP - RA
    SA = int(sum(KA)) // SCC         # segment-A supers (padded to %SCC)
    MT = NP // 128

    t_of_q0 = []
    for t in range(NT):
        t_of_q0 += [t] * K0[t]
    t_of_q1 = []
    for t in range(NT):
        t_of_q1 += [t] * KA[t]
    for t in range(NT):
        t_of_q1 += [t] * KB[t]

    nc = bacc.Bacc(dynamic_dma_scratch_size=65536, num_swdge_queues=4)
    P = nc.declare_dram_parameter

    xT = P("xT", [F_IN, NP], BF16, isOutput=False)
    r0h = P("r0h", [F_IN, HC], BF16, isOutput=False)
    r0a = P("r0a", [F_IN, 8], BF16, isOutput=False)
    r1h = P("r1h", [HC, HC], BF16, isOutput=False)
    r1a = P("r1a", [HC, 8], BF16, isOutput=False)
    r2 = P("r2", [HC, FTS], F32, isOutput=False)
    r3 = P("r3", [FTS, 1], F32, isOutput=False)
    ident = P("ident", [128, 128], F32, isOutput=False)
    identb = P("identb", [128, 128], BF16, isOutput=False)
    srcw0 = P("srcw0", [128, SW0], I16, isOutput=False)
    srcw1 = P("srcw1", [128, SW1], I16, isOutput=False)
    trw = P("trw", [128, NT * 8], I16, isOutput=False)
    ohz0 = P("ohz0", [128, NC0, 256], F8, isOutput=False)
    ohz1 = P("ohz1", [128, NC1, 256], F8, isOutput=False)
    ae0 = P("ae0", [128, NC0, 4], F32, isOutput=False)
    ae1 = P("ae1", [128, NC1, 4], F32, isOutput=False)
    if use_b0:
        b0t = P("b0t", [128, HC], F32, isOutput=False)
    if use_b1:
        b1t = P("b1t", [128, HC], F32, isOutput=False)
    if use_l0b:
        l0bt = P("l0bt", [128, FTS], F32, isOutput=False)
    if use_l1b:
        l1bt = P("l1bt", [128, 1], F32, isOutput=False)
    outp = P("out", [NT * 128, 1], F32, isOutput=True)

    with tile.TileContext(nc) as tc:
        with (
            tc.tile_pool(name="const", bufs=1) as const,
            tc.tile_pool(name="stage", bufs=2) as stage,
            tc.tile_pool(name="work", bufs=3) as work,
            tc.tile_pool(name="tp", bufs=6) as tp,
            tc.tile_pool(name="adp", bufs=1) as adp,
            tc.tile_pool(name="psacc", bufs=3, space="PSUM") as psacc,
            tc.tile_pool(name="pss", bufs=2, space="PSUM") as pss,
            tc.tile_pool(name="pstr", bufs=3, space="PSUM") as pstr,
            tc.tile_pool(name="dram", bufs=1, space="DRAM") as dram,
        ):
            H0 = dram.tile([NP, TW], BF16, tag="H0")
            H1A = dram.tile([RA, TW], BF16, tag="H1A", addr_space="Shared")
            H1B = dram.tile([RB, TW], BF16, tag="H1B", addr_space="Shared")
            H1g = [dram.tile([(NTA if g == 0 else NTB) * 128, TW], BF16,
                             tag=f"H1g{g}", name=f"H1g{g}") for g in range(2)]

            _cn = [0]

            def cload(ap_in, shape, dt=F32, tag=None):
                _cn[0] += 1
                cname = tag or f"c{_cn[0]}"
                t = const.tile(shape, dt, tag=cname, name=f"{cname}_{_cn[0]}")
                nc.sync.dma_start(out=t[:], in_=ap_in)
                return t

            r0h_s = cload(r0h[:, :], [F_IN, HC], BF16)
            r0a_s = cload(r0a[:, :], [F_IN, 8], BF16)
            r1h_s = [cload(r1h[k * 128:(k + 1) * 128, :], [128, HC], BF16)
                     for k in range(KB_)]
            r1a_s = [cload(r1a[k * 128:(k + 1) * 128, :], [128, 8], BF16)
                     for k in range(KB_)]
            r2_s = [cload(r2[k * 128:(k + 1) * 128, :], [128, FTS])
                    for k in range(KB_)]
            r3_s = cload(r3[:, :], [FTS, 1])
            id_s = cload(ident[:, :], [128, 128])
            idb_s = cload(identb[:, :], [128, 128], BF16)
            srcw0_s = cload(srcw0[:, :], [128, SW0], I16, tag="srcw")
            trw_s = cload(trw[:, :], [128, NT * 8], I16)
            ae0_s = cload(ae0[:, :, :], [128, NC0, 4], tag="ae")
            b0_s = cload(b0t[:, :], [128, HC]) if use_b0 else None
            b1_s = cload(b1t[:, :], [128, HC]) if use_b1 else None
            l0b_s = cload(l0bt[:, :], [128, FTS]) if use_l0b else None
            l1b_s = cload(l1bt[:, :], [128, 1]) if use_l1b else None

            # ---- phase A: layer-0 table (xT pre-permuted to row order) ----
            nc._state.push_named_scope("phaseA")
            NQ4 = NP // 4
            MQ = MT // 4
            xq = [None] * 4

            def load_xq(qq):
                if qq >= 4:
                    return
                xq[qq] = stage.tile([F_IN, NQ4], BF16, tag="xT",
                                    name=f"xq{qq}", bufs=2)
                eng = nc.sync if qq < 2 else nc.scalar
                eng.dma_start(out=xq[qq][:],
                              in_=xT[:, qq * NQ4:(qq + 1) * NQ4])

            load_xq(0)
            load_xq(1)
            st2 = None
            for mr in range(MT):
                if mr % MQ == MQ - 1:
                    load_xq(mr // MQ + 2)
                lx = xq[mr // MQ][:, (mr % MQ) * 128:(mr % MQ + 1) * 128]
                pool = psacc if mr % 2 == 0 else pstr
                ph = pool.tile([128, HC], F32,
                               tag="ph" if mr % 2 == 0 else "pt", name="phA")
                nc.tensor.matmul(ph[:], lx, r0h_s[:], start=True, stop=True)
                pa = pss.tile([128, 8], F32, tag="pss")
                nc.tensor.matmul(pa[:], lx, r0a_s[:], start=True, stop=True)
                half = mr % 2
                if half == 0:
                    st2 = stage.tile([128, 2, TW], BF16, tag="hrow2", bufs=2)
                if half == 0:
                    nc.vector.tensor_copy(st2[:, half, 0:HC], ph[:])
                    nc.vector.tensor_copy(
                        st2[:, half, HC:HC + 16].bitcast(F32), pa[:])
                else:
                    nc.scalar.activation(st2[:, half, 0:HC], ph[:], AF.Copy)
                    nc.vector.tensor_copy(
                        st2[:, half, HC:HC + 16].bitcast(F32), pa[:])
                if half == 1:
                    dst = H0[(mr - 1) * 128:(mr + 1) * 128, :].rearrange(
                        "(j p) c -> p j c", p=128)
                    eng = nc.sync if (mr // 2) % 2 == 0 else nc.scalar
                    eng.dma_start(out=dst, in_=st2[:])
            nc._state.pop_named_scope("phaseA")

            # ---- layer-0 alpha_dst for own dst tiles: one batched gather ----
            adt_all = adp.tile([128, NT, 128], BF16, tag="adta")
            nc.gpsimd.dma_gather(
                adt_all[:], H0[:, HC:TW], trw_s[:, 0:NT * 8],
                NT * 128, NT * 128, 128, elem_step=TW,
                single_packet=False, queue_num=3)
            adts0 = adp.tile([128, NT, 4], BF16, tag="adts0")
            nc.vector.tensor_copy(adts0[:], adt_all[:, :, 8:16].bitcast(F32))

            # persistent alpha_dst for layer-1's own dst tiles (filled in fin0)
            adts1 = adp.tile([128, NT, 4], BF16, tag="adts1")
            # SBUF partial-sum parking for layer-1's two-pass aggregation
            accO = adp.tile([128, NT, HC], F32, tag="adta")
            accS = adp.tile([128, NT, 4], F32, tag="accS")

            # ---- aggregation over one layer's edges ----
            def agg_layer(tbl_of_s, srcw_s, ohz_p, ae_s, adts, t_of_q,
                          NCHUNK, passes, fin_pre, fin_post, acc_last):
                NSUP = -(-NCHUNK // SCC)
                gstate = {}

                def nch_of(s):
                    return min(SCC, NCHUNK - s * SCC)

                def issue_gather(s):
                    if s >= NSUP:
                        return
                    nch = nch_of(s)
                    tbl = tbl_of_s(s)
                    gA = stage.tile([128, SCC, TW], BF16, tag="gA",
                                    name="gA", bufs=4)
                    c0 = s * SCC * 8
                    base = 0
                    nq = min(4, nch)
                    for qi in range(nq):
                        take = (nch - base + (nq - qi) - 1) // (nq - qi)
                        nc.gpsimd.dma_gather(
                            gA[:, base:base + take, :], tbl,
                            srcw_s[:, c0 + base * 8:c0 + (base + take) * 8],
                            take * 128, take * 128, TW,
                            single_packet=False, queue_num=qi)
                        base += take
                    oz = stage.tile([128, SCC, 256], F8, tag="ohz",
                                    name="oz", bufs=4)
                    nc.sync.dma_start(
                        out=oz[:, 0:nch, :],
                        in_=ohz_p[:, s * SCC:s * SCC + nch, :])
                    gstate[s] = [gA, oz, None, None]

                def emit_pead(s):
                    if s >= NSUP:
                        return
                    nch = nch_of(s)
                    oz = gstate[s][1]
                    pead = pstr.tile([128, SCC * 4], F32, tag="pt", name="pead")
                    for jj in range(nch):
                        q = s * SCC + jj
                        nc.tensor.matmul(
                            pead[:, jj * 4:(jj + 1) * 4],
                            oz[:, jj, 128:256], adts[:, t_of_q[q], :],
                            start=True, stop=True)
                    gstate[s][2] = pead

                def emit_alpha(s):
                    if s >= NSUP:
                        return
                    nch = nch_of(s)
                    gA, oz, pead, _ = gstate[s]
                    asrc = gA[:, 0:nch, HC:HC + 8].bitcast(F32)
                    t0 = work.tile([128, SCC, 4], F32, tag="t0", bufs=2)
                    nc.vector.tensor_add(
                        t0[:, 0:nch, :], asrc,
                        ae_s[:, s * SCC:s * SCC + nch, :])
                    t1 = work.tile([128, SCC, 4], F32, tag="t1", bufs=2)
                    nc.vector.tensor_add(
                        t1[:, 0:nch, :], t0[:, 0:nch, :],
                        pead[:, 0:nch * 4].rearrange("x (a b) -> x a b", b=4))
                    t2 = work.tile([128, SCC, 4], F32, tag="t2", bufs=2)
                    nc.scalar.activation(
                        t2[:, 0:nch, :], t1[:, 0:nch, :], AF.Copy, scale=0.2)
                    tl = work.tile([128, SCC, 4], F32, tag="tl", bufs=2)
                    nc.vector.tensor_max(
                        tl[:, 0:nch, :], t1[:, 0:nch, :], t2[:, 0:nch, :])
                    pb = work.tile([128, SCC, 4], BF16, tag="p", bufs=3)
                    nc.scalar.activation(pb[:, 0:nch, :], tl[:, 0:nch, :],
                                         AF.Exp)
                    gstate[s][3] = pb

                issue_gather(0)
                issue_gather(1)
                emit_pead(0)
                emit_alpha(0)

                pend = None
                q = 0
                last_pass = len(passes) - 1
                for pi, seg in enumerate(passes):
                    final = (pi == last_pass)
                    for (t, cnt) in seg:
                        ps_o = psacc.tile([128, HC], F32, tag="ph")
                        ps_s = pss.tile([128, 8], F32, tag="pss")
                        fin_t, fin_ar = None, None
                        for k in range(cnt):
                            s, j = divmod(q, SCC)
                            if j == 0:
                                issue_gather(s + 2)
                                emit_pead(s + 1)
                            if j == 1:
                                emit_alpha(s + 1)
                            if final and pend is not None and \
                                    k == min(2, cnt - 1):
                                fin_t = pend[0]
                                fin_ar = fin_pre(*pend)
                                pend = None
                            gA, oz, pead, pb = gstate[s]
                            gp = work.tile([128, HC], BF16, tag="gp", bufs=4)
                            nc.vector.tensor_mul(
                                gp[:].rearrange("x (c h) -> x c h", h=H),
                                gA[:, j, 0:HC].rearrange(
                                    "x (c h) -> x c h", h=H),
                                _bcastI(pb, j, C))
                            first, last = (k == 0), (k == cnt - 1)
                            oh_j = oz[:, j, 0:128]
                            nc.tensor.matmul(ps_o[:], oh_j, gp[:],
                                             start=first, stop=last)
                            nc.tensor.matmul(ps_s[:, 0:4], oh_j, pb[:, j, :],
                                             start=first, stop=last)
                            q += 1
                        if not final:
                            # park partial sums in SBUF
                            if t % 2 == 0:
                                nc.vector.tensor_copy(accO[:, t, :], ps_o[:])
                            else:
                                nc.scalar.activation(accO[:, t, :], ps_o[:],
                                                     AF.Copy)
                            nc.vector.tensor_copy(accS[:, t, :], ps_s[:, 0:4])
                        else:
                            if fin_ar is not None:
                                fin_post(fin_t, fin_ar)
                            pend = (t, ps_o, ps_s)
                ar = fin_pre(*pend)
                fin_post(pend[0], ar)

            # ---- tile finalize: softmax-normalize + relu (fused on scalar) --
            def norm_relu(ps_o, ps_s, acc_t, bias_s, out_dt):
                if acc_t is not None:
                    so = work.tile([128, HC], F32, tag="so", bufs=2)
                    nc.vector.tensor_add(so[:], ps_o[:], accO[:, acc_t, :])
                    ss = work.tile([128, 4], F32, tag="ss", bufs=2)
                    nc.vector.tensor_add(ss[:], ps_s[:, 0:4],
                                         accS[:, acc_t, :])
                else:
                    so, ss = ps_o, ps_s[:, 0:4]
                sp = work.tile([128, 4], F32, tag="sp")
                nc.vector.tensor_scalar_add(sp[:], ss[:], 1e-16)
                rc = work.tile([128, 4], F32, tag="rc")
                nc.vector.reciprocal(rc[:], sp[:])
                if bias_s is None:
                    ar = work.tile([128, HC], out_dt,
                                   tag=f"ar{out_dt}", bufs=2)
                    arv = ar[:].rearrange("x (c h) -> x h c", h=H)
                    psv = so[:].rearrange("x (c h) -> x h c", h=H)
                    for h in range(H):
                        nc.scalar.activation(
                            arv[:, h, :], psv[:, h, :],
                            AF.Relu, scale=rc[:, h:h + 1])
                    return ar
                ao = work.tile([128, HC], F32, tag="ao", bufs=2)
                aov = ao[:].rearrange("x (c h) -> x h c", h=H)
                psv = so[:].rearrange("x (c h) -> x h c", h=H)
                for h in range(H):
                    nc.vector.tensor_scalar_mul(
                        aov[:, h, :], psv[:, h, :], rc[:, h:h + 1])
                ab = work.tile([128, HC], F32, tag="ao", bufs=2)
                nc.vector.tensor_add(ab[:], ao[:], bias_s[:])
                ar = work.tile([128, HC], out_dt, tag=f"ar{out_dt}", bufs=2)
                nc.scalar.activation(ar[:], ab[:], AF.Relu)
                return ar

            # ---- layer-0 finalize: transpose + layer-1 linear + half AG ----
            def fin_pre0(t, ps_o, ps_s):
                return norm_relu(ps_o, ps_s, None, b0_s, BF16)

            def fin_post0(t, ar):
                a0k = []
                for kk in range(KB_):
                    pt = pstr.tile([128, 128], BF16, tag="pt", name="ptb")
                    nc.tensor.transpose(pt[:], ar[:, kk * 128:(kk + 1) * 128],
                                        idb_s[:])
                    ak = tp.tile([128, 128], BF16, tag="a1T", name=f"a0k{kk}")
                    if kk % 2 == 0:
                        nc.scalar.activation(ak[:], pt[:], AF.Copy)
                    else:
                        nc.vector.tensor_copy(ak[:], pt[:])
                    a0k.append(ak)
                ph1 = psacc.tile([128, HC], F32, tag="ph")
                pa1 = pss.tile([128, 8], F32, tag="pss")
                for kk in range(KB_):
                    first, last = (kk == 0), (kk == KB_ - 1)
                    nc.tensor.matmul(ph1[:], a0k[kk][:], r1h_s[kk][:],
                                     start=first, stop=last)
                    nc.tensor.matmul(pa1[:], a0k[kk][:], r1a_s[kk][:],
                                     start=first, stop=last)
                st = stage.tile([128, TW], BF16, tag="hrow", bufs=3)
                if t % 2 == 0:
                    nc.vector.tensor_copy(st[:, 0:HC], ph1[:])
                else:
                    nc.scalar.activation(st[:, 0:HC], ph1[:], AF.Copy)
                nc.vector.tensor_copy(st[:, HC:HC + 16].bitcast(F32), pa1[:])
                nc.vector.tensor_copy(adts1[:, t, :], pa1[:, 4:8])
                g = 0 if t < NTA else 1
                loc = t - g * NTA
                nc.sync.dma_start(out=H1g[g][loc * 128:(loc + 1) * 128, :],
                                  in_=st[:])
                if loc == (NTA if g == 0 else NTB) - 1:
                    nc.gpsimd.collective_compute(
                        "AllGather", OP.bypass,
                        replica_groups=[list(range(NCORES))],
                        ins=[H1g[g].opt()],
                        outs=[(H1A if g == 0 else H1B)[:, :].opt()],
                    )

            nc._state.push_named_scope("phaseB")
            agg_layer(lambda s: H0[:, :], srcw0_s, ohz0, ae0_s, adts0,
                      t_of_q0, NC0, [[(t, K0[t]) for t in range(NT)]],
                      fin_pre0, fin_post0, None)
            nc._state.pop_named_scope("phaseB")

            # ---- layer-1 aggregation + MLP head per dst tile ----
            def fin_pre1(t, ps_o, ps_s):
                return norm_relu(ps_o, ps_s, t, b1_s, F32)

            def fin_post1(t, ar):
                h2p = psacc.tile([128, FTS], F32, tag="ph")
                for kk in range(KB_):
                    pt = pstr.tile([128, 128], F32, tag="pt", name="ptf")
                    nc.tensor.transpose(pt[:], ar[:, kk * 128:(kk + 1) * 128],
                                        id_s[:])
                    a1k = tp.tile([128, 128], F32, tag="a1T")
                    nc.vector.tensor_copy(a1k[:], pt[:])
                    nc.tensor.matmul(h2p[:], a1k[:], r2_s[kk][:],
                                     start=(kk == 0), stop=(kk == KB_ - 1))
                if use_l0b:
                    h2b = work.tile([128, FTS], F32, tag="h2b")
                    nc.vector.tensor_add(h2b[:], h2p[:], l0b_s[:])
                else:
                    h2b = h2p
                h2r = work.tile([128, FTS], F32, tag="h2r")
                nc.scalar.activation(h2r[:], h2b[:], AF.Relu)
                pt2 = pstr.tile([128, 128], F32, tag="pt")
                nc.tensor.transpose(pt2[:], h2r[:], id_s[:])
                h2T = tp.tile([128, 128], F32, tag="a1T")
                nc.vector.tensor_copy(h2T[:], pt2[:])
                po = pss.tile([128, 8], F32, tag="pss")
                nc.tensor.matmul(po[:, 0:1], h2T[:], r3_s[:],
                                 start=True, stop=True)
                ob = work.tile([128, 1], F32, tag="ob")
                if use_l1b:
                    nc.vector.tensor_add(ob[:], po[:, 0:1], l1b_s[:])
                else:
                    nc.vector.tensor_copy(ob[:], po[:, 0:1])
                nc.sync.dma_start(out=outp[t * 128:(t + 1) * 128, :], in_=ob[:])

            nc._state.push_named_scope("phaseD")
            srcw1_s = cload(srcw1[:, :], [128, SW1], I16, tag="srcw")
            ae1_s = cload(ae1[:, :, :], [128, NC1, 4], tag="ae")
            agg_layer(lambda s: H1A[:, :] if s < SA else H1B[:, :],
                      srcw1_s, ohz1, ae1_s, adts1, t_of_q1, NC1,
                      [[(t, KA[t]) for t in range(NT)],
                       [(t, KB[t]) for t in range(NT)]],
                      fin_pre1, fin_post1, True)
            nc._state.pop_named_scope("phaseD")

    nc.finalize()
    return nc


def _wrap_idx(v, E_pad):
    blk = np.zeros((16, E_pad // 16), np.int16)
    ar = np.arange(E_pad)
    blk[ar % 16, ar // 16] = v.astype(np.int16)
    return np.tile(blk, (8, 1))


def kernel(x, edge_index, edge_weights,
           W0, as0, ad0, We0, ae0, b0,
           W1, as1, ad1, We1, ae1, b1,
           L0W, L0b, L1W, L1b):
    x = np.asarray(x, np.float32)
    N, F_IN = x.shape
    HC = W0.shape[0]
    H, C = np.asarray(as0).shape
    FTS = np.asarray(L0W).shape[0]

    NT = -(-N // (128 * NCORES))
    NTA = NT // 2
    NTB = NT - NTA
    SHARD = NT * 128
    NP = SHARD * NCORES
    RA = NCORES * NTA * 128

    # table-row permutation: half-major (A=tiles<NTA), core-interleaved within
    nodes = np.arange(NP)
    core = nodes // SHARD
    rr = nodes % SHARD
    tt = rr // 128
    gg = (tt >= NTA).astype(np.int64)
    off = (tt - gg * NTA) * 128 + rr % 128
    stride = np.where(gg == 0, NTA * 128, NTB * 128)
    t_of_n = gg * RA + core * stride + off            # node -> table row

    # ---- edges ----
    ew_in = np.asarray(edge_weights, np.float32)
    src = np.concatenate([np.asarray(edge_index[0]), np.arange(N)])
    dst = np.concatenate([np.asarray(edge_index[1]), np.arange(N)])
    ew = np.concatenate([ew_in, np.full(N, ew_in.mean(), np.float32)])
    order = np.argsort(dst, kind="stable")
    src_s, dst_s, ew_s = src[order], dst[order], ew[order]
    srow = t_of_n[src_s]                              # src table row

    NTG = NP // 128
    tile_of = (dst_s // 128).astype(np.int64)
    tcounts = np.bincount(tile_of, minlength=NTG)
    tstart = np.concatenate([[0], np.cumsum(tcounts)])

    # layer-0 chunk counts (per dst tile, max over cores)
    K0 = [max(1, int(max(-(-tcounts[i * NT + t] // 128)
                         for i in range(NCORES)))) for t in range(NT)]
    NC0 = int(sum(K0))

    # layer-1 per-tile per-src-half counts
    cA = np.zeros((NCORES, NT), np.int64)
    cB = np.zeros((NCORES, NT), np.int64)
    for i in range(NCORES):
        for t in range(NT):
            g = i * NT + t
            sl = slice(tstart[g], tstart[g] + int(tcounts[g]))
            inA = srow[sl] < RA
            cA[i, t] = int(inA.sum())
            cB[i, t] = int((~inA).sum())
    KA = [max(1, int(max(-(-cA[i, t] // 128) for i in range(NCORES))))
          for t in range(NT)]
    KB = [max(1, int(max(-(-cB[i, t] // 128) for i in range(NCORES))))
          for t in range(NT)]
    KA[NT - 1] += (-sum(KA)) % SCC     # segment-A supers end on a boundary
    NC1 = int(sum(KA) + sum(KB))

    # ---- weight folding (host, O(weights)) ----
    as0 = np.asarray(as0, np.float32)
    ad0 = np.asarray(ad0, np.float32)
    ae0w = np.asarray(ae0, np.float32)
    as1 = np.asarray(as1, np.float32)
    ad1 = np.asarray(ad1, np.float32)
    ae1w = np.asarray(ae1, np.float32)
    W0 = np.asarray(W0, np.float32)
    W1 = np.asarray(W1, np.float32)
    We0 = np.asarray(We0, np.float32)
    We1 = np.asarray(We1, np.float32)

    k0 = (We0.reshape(H, C) * ae0w).sum(1).astype(np.float32)
    k1 = (We1.reshape(H, C) * ae1w).sum(1).astype(np.float32)

    def fold(W, a):
        blk = np.zeros((HC, H), np.float32)
        for h in range(H):
            blk[h * C:(h + 1) * C, h] = a[h]
        return (W.T @ blk).astype(np.float32)

    bf = ml_dtypes.bfloat16
    f8 = ml_dtypes.float8_e4m3fn
    # head-interleaved column order: table col c*H+h holds head h channel c
    iperm = np.asarray([h * C + c for c in range(C) for h in range(H)])
    r0h = W0.T[:, iperm].astype(bf)
    r0a = np.concatenate([fold(W0, as0), fold(W0, ad0)], 1).astype(bf)
    r1h = W1.T[iperm][:, iperm].astype(bf)
    r1a = np.concatenate([fold(W1, as1), fold(W1, ad1)], 1)[iperm].astype(bf)
    r2 = np.asarray(L0W, np.float32).T[iperm].copy()
    r3 = np.asarray(L1W, np.float32).T.copy()

    # xT in TABLE-ROW order: column r of xT = x[node(r)]
    inv = np.empty(NP, np.int64)
    inv[t_of_n] = nodes                              # table row -> node
    xa = np.zeros((NP, F_IN), np.float32)
    xa[:N] = x
    xT = np.ascontiguousarray(xa[inv].T).astype(bf)

    ident = np.eye(128, dtype=np.float32)
    identb = np.eye(128, dtype=np.float32).astype(bf)

    use_b0 = bool(np.any(b0))
    use_b1 = bool(np.any(b1))
    use_l0b = bool(np.any(np.asarray(L0b)))
    use_l1b = bool(np.any(np.asarray(L1b)))

    def build_oh(dlocp, nchunk):
        ohcube = np.zeros((nchunk, 128, 128), np.float32)  # [q, e, d]
        dl2 = dlocp.reshape(nchunk, 128)
        qs, es = np.nonzero(dl2 >= 0)
        ohcube[qs, es, dl2[qs, es]] = 1.0
        ohz_np = np.empty((128, nchunk, 256), f8)
        ohz_np[:, :, 0:128] = ohcube.transpose(1, 0, 2).astype(f8)
        ohz_np[:, :, 128:256] = ohcube.transpose(2, 0, 1).astype(f8)
        return ohz_np

    in_maps = []
    for i in range(NCORES):
        # ---- layer-0 edge layout: per-tile contiguous ----
        E0 = NC0 * 128
        srcp0 = np.zeros(E0, np.int64)
        dloc0 = np.full(E0, -1, np.int64)
        ewp0 = np.zeros(E0, np.float32)
        offq = 0
        for t in range(NT):
            g = i * NT + t
            cnt = int(tcounts[g])
            sl = slice(tstart[g], tstart[g] + cnt)
            srcp0[offq:offq + cnt] = srow[sl]
            dloc0[offq:offq + cnt] = dst_s[sl] - g * 128
            ewp0[offq:offq + cnt] = ew_s[sl]
            offq += K0[t] * 128
        # ---- layer-1 edge layout: [tiles x half-A | tiles x half-B] ----
        E1 = NC1 * 128
        srcp1 = np.zeros(E1, np.int64)
        dloc1 = np.full(E1, -1, np.int64)
        ewp1 = np.zeros(E1, np.float32)
        offq = 0
        for seg in (0, 1):
            Ks = KA if seg == 0 else KB
            for t in range(NT):
                g = i * NT + t
                sl = slice(tstart[g], tstart[g] + int(tcounts[g]))
                inA = srow[sl] < RA
                pick = inA if seg == 0 else ~inA
                cnt = int(pick.sum())
                rows = srow[sl][pick]
                if seg == 1:
                    rows = rows - RA
                srcp1[offq:offq + cnt] = rows
                dloc1[offq:offq + cnt] = dst_s[sl][pick] - g * 128
                ewp1[offq:offq + cnt] = ew_s[sl][pick]
                offq += Ks[t] * 128
        ae0p = (ewp0[:, None] * k0[None, :]).reshape(NC0, 128, 4)
        ae1p = (ewp1[:, None] * k1[None, :]).reshape(NC1, 128, 4)
        # own dst-tile table rows for the layer-0 alpha_dst gather
        trows = np.empty((NT, 128), np.int64)
        for t in range(NT):
            base = t_of_n[i * SHARD + t * 128]
            trows[t] = base + np.arange(128)
        im = {
            "xT": xT, "r0h": r0h, "r0a": r0a, "r1h": r1h, "r1a": r1a,
            "r2": r2, "r3": r3, "ident": ident, "identb": identb,
            "srcw0": _wrap_idx(srcp0, E0), "srcw1": _wrap_idx(srcp1, E1),
            "trw": _wrap_idx(trows.reshape(-1), NT * 128),
            "ohz0": build_oh(dloc0, NC0), "ohz1": build_oh(dloc1, NC1),
            "ae0": np.ascontiguousarray(ae0p.transpose(1, 0, 2)),
            "ae1": np.ascontiguousarray(ae1p.transpose(1, 0, 2)),
        }
        if use_b0:
            im["b0t"] = np.tile(np.asarray(b0, np.float32)[iperm][None, :],
                                (128, 1))
        if use_b1:
            im["b1t"] = np.tile(np.asarray(b1, np.float32)[iperm][None, :],
                                (128, 1))
        if use_l0b:
            im["l0bt"] = np.tile(np.asarray(L0b, np.float32)[None, :],
                                 (128, 1))
        if use_l1b:
            im["l1bt"] = np.tile(np.asarray(L1b, np.float32).reshape(1, 1),
                                 (128, 1))
        in_maps.append(im)

    nc = _build_program(NP, F_IN, HC, H, C, NT, K0, KA, KB, NTA, FTS,
                        use_b0, use_b1, use_l0b, use_l1b)
    res = run_bass_kernel_spmd(nc, in_maps, list(range(NCORES)))
    out = np.concatenate([res.results[i]["out"][:, 0] for i in range(NCORES)])
    return out[:N].astype(np.float32)
